# revision 3
# baseline (speedup 1.0000x reference)
"""MLA decode kernel for Trainium2, 8-way head-sharded (tensor parallel).

Math (weight-absorbed MLA decode, per head h):
  q_eff = q_C @ W_UK_h^T                      # fold k up-projection into q
  score[s] = q_eff . c_KV[s] + q_R . k_R[s]   # latent-space attention
  attn_lat = softmax(score) @ c_KV            # latent-space PV
  out_h = attn_lat @ W_UV_h                   # up-project once per head
This avoids materializing the [B,S,H,D] K/V tensors (the naive 137 GFLOP path)
and makes the kernel memory-bound (~84 MiB/core HBM traffic).

Each core handles 4 of 32 heads for all 4 batches; the latent cache and the
down-projection weights are replicated, the up/out projections are sliced by
head. Host sums the 8 partial W_O outputs and assembles the new caches.
"""

import sys
import os
import math

for _p in ("/opt/trn_rl_repo", "/root/.axon_site/_ro/trn_rl_repo"):
    if os.path.isdir(_p) and _p not in sys.path:
        sys.path.insert(0, _p)

import numpy as np

import concourse.bass as bass
import concourse.mybir as mybir
import concourse.tile as tile
from concourse import bacc
from concourse.bass_utils import run_bass_kernel_spmd

F32 = mybir.dt.float32
AF = mybir.ActivationFunctionType
OP = mybir.AluOpType

NUM_HEADS = 32
HEAD_DIM = 128
ROPE_DIM = 64
HIDDEN = 4096
C_KV = 512
C_Q = 1536
CACHE_LEN = 4096
BATCH = 4
N_CORES = 8
H_LOC = NUM_HEADS // N_CORES          # 4 heads per core
SCALE = 1.0 / math.sqrt(HEAD_DIM + ROPE_DIM)

NSUP = CACHE_LEN // 512               # 8 seq super-chunks of 512


def _build_nc():
    nc = bacc.Bacc()

    hid_sw = nc.dram_tensor("hid_sw", (128, 128), F32, kind="ExternalInput")
    ckv_c = nc.dram_tensor("ckv_c", (BATCH, CACHE_LEN, C_KV), F32, kind="ExternalInput")
    krT = nc.dram_tensor("krT", (BATCH, ROPE_DIM, CACHE_LEN), F32, kind="ExternalInput")
    mask_s = nc.dram_tensor("mask_s", (BATCH, CACHE_LEN + 1), F32, kind="ExternalInput")
    mask_c = nc.dram_tensor("mask_c", (1, BATCH), F32, kind="ExternalInput")
    wdkv = nc.dram_tensor("wdkv", (HIDDEN, C_KV), F32, kind="ExternalInput")
    wdq = nc.dram_tensor("wdq", (HIDDEN, C_Q), F32, kind="ExternalInput")
    wkr = nc.dram_tensor("wkr", (HIDDEN, ROPE_DIM), F32, kind="ExternalInput")
    wuq_s = nc.dram_tensor("wuq_s", (C_Q, H_LOC * HEAD_DIM), F32, kind="ExternalInput")
    wqr_s = nc.dram_tensor("wqr_s", (C_Q, H_LOC * ROPE_DIM), F32, kind="ExternalInput")
    wukt_s = nc.dram_tensor("wukt_s", (H_LOC * HEAD_DIM, C_KV), F32, kind="ExternalInput")
    wuv_s = nc.dram_tensor("wuv_s", (C_KV, H_LOC * HEAD_DIM), F32, kind="ExternalInput")
    wo_s = nc.dram_tensor("wo_s", (H_LOC * HEAD_DIM, HIDDEN), F32, kind="ExternalInput")
    cos_q = nc.dram_tensor("cos_q", (BATCH, 128), F32, kind="ExternalInput")
    sin_q = nc.dram_tensor("sin_q", (BATCH, 128), F32, kind="ExternalInput")
    cos_k = nc.dram_tensor("cos_k", (BATCH, 32), F32, kind="ExternalInput")
    sin_k = nc.dram_tensor("sin_k", (BATCH, 32), F32, kind="ExternalInput")
    ident = nc.dram_tensor("ident", (128, 128), F32, kind="ExternalInput")

    out_p = nc.dram_tensor("out_p", (BATCH, HIDDEN), F32, kind="ExternalOutput")
    ckv_new = nc.dram_tensor("ckv_new", (BATCH, C_KV), F32, kind="ExternalOutput")
    kr_new = nc.dram_tensor("kr_new", (BATCH, ROPE_DIM), F32, kind="ExternalOutput")

    with tile.TileContext(nc) as tc:
        with tc.tile_pool(name="persist", bufs=1) as pp:
            id_sb = pp.tile([128, 128], F32)
            nc.sync.dma_start(id_sb[:], ident[:, :])
            id4 = id_sb[0:4, 0:4]

            # probs for all (b@32|h) rows x 4224 (33 chunks of 128) columns
            probs_sb = pp.tile([128, 4224], F32)
            nc.vector.memset(probs_sb[:], 0.0)

            qeffT_sb = pp.tile([128, 4, 4, 4], F32)     # [c_part, cb, h, b]
            qaug4_sb = pp.tile([65, 4, 4], F32)         # [rope+1, h, b]
            nc.vector.memset(qaug4_sb[64:65, :, :], 1.0)
            ckvT_sb = pp.tile([128, 4, 4], F32)         # [c_part, cb, b]
            kRcur_sb = pp.tile([65, 4], F32)            # [rope+mask, b]
            ckv_sb = pp.tile([4, 512], F32)
            kr_sb = pp.tile([4, 64], F32)
            ccur_sb = pp.tile([4, 4, 512], F32)         # K=4 padded current-token V rows
            nc.vector.memset(ccur_sb[:], 0.0)
            pT33_sb = pp.tile([4, 128], F32)            # probs^T of current token (rows 1-3 zero)
            den_sb = pp.tile([128, 1], F32)
            recip_sb = pp.tile([128, 1], F32)
            attn_sb = pp.tile([128, 512], F32)
            attnT_sb = pp.tile([128, 4, 4, 32], F32)    # [c_part, cb, b, h]
            ohT_sb = pp.tile([128, 4, 4], F32)          # [d, h, b]
            outp_sb = pp.tile([4, 4096], F32)

            # ---------------- Phase 0: projections -------------------------
            with tc.tile_pool(name="p0c", bufs=1) as p0c, \
                 tc.tile_pool(name="p0s", bufs=3) as p0s, \
                 tc.tile_pool(name="p0ps", bufs=1, space="PSUM") as p0ps:
                hid_sb = p0c.tile([128, 32, 4], F32)
                nc.sync.dma_start(hid_sb[:], hid_sw[:, :].rearrange("p (kc b) -> p kc b", b=4))
                wuq_sb = p0c.tile([128, 12, 512], F32)
                nc.sync.dma_start(wuq_sb[:], wuq_s[:, :].rearrange("(kc p) n -> p kc n", p=128))
                wqr_sb = p0c.tile([128, 12, 256], F32)
                nc.sync.dma_start(wqr_sb[:], wqr_s[:, :].rearrange("(kc p) n -> p kc n", p=128))
                wukt_sb = p0c.tile([128, 4, 512], F32)
                nc.sync.dma_start(wukt_sb[:], wukt_s[:, :].rearrange("(kb p) c -> p kb c", p=128))
                cosq_sb = p0c.tile([4, 128], F32)
                nc.sync.dma_start(cosq_sb[:], cos_q[:, :])
                sinq_sb = p0c.tile([4, 128], F32)
                nc.sync.dma_start(sinq_sb[:], sin_q[:, :])
                cosk_sb = p0c.tile([4, 32], F32)
                nc.sync.dma_start(cosk_sb[:], cos_k[:, :])
                sink_sb = p0c.tile([4, 32], F32)
                nc.sync.dma_start(sink_sb[:], sin_k[:, :])
                cq_sb = p0c.tile([4, 1536], F32)
                qc_sb = p0c.tile([4, 512], F32)
                qr_sb = p0c.tile([4, 256], F32)
                cqT_sb = p0c.tile([128, 12, 4], F32)
                qcT_sb = p0c.tile([128, 4, 4], F32)

                # c_Q = hidden @ W_DQ ; c_KV = hidden @ W_DKV ; k_R = hidden @ W_KR
                cq_ps = p0ps.tile([4, 1536], F32, tag="cq")
                ckv_ps = p0ps.tile([4, 512], F32, tag="ckv")
                kr_ps = p0ps.tile([4, 256], F32, tag="kr", name="kr_ps")[:, 0:64]
                for kc in range(32):
                    wdq_t = p0s.tile([128, 1536], F32, tag="wdq")
                    nc.sync.dma_start(wdq_t[:], wdq[kc * 128:(kc + 1) * 128, :])
                    wdkv_t = p0s.tile([128, 512], F32, tag="wdkv")
                    nc.sync.dma_start(wdkv_t[:], wdkv[kc * 128:(kc + 1) * 128, :])
                    wkr_t = p0s.tile([128, 64], F32, tag="wkr")
                    nc.sync.dma_start(wkr_t[:], wkr[kc * 128:(kc + 1) * 128, :])
                    st = kc == 0
                    sp = kc == 31
                    for n in range(3):
                        nc.tensor.matmul(cq_ps[:, n * 512:(n + 1) * 512], hid_sb[:, kc, :],
                                         wdq_t[:, n * 512:(n + 1) * 512], start=st, stop=sp)
                    nc.tensor.matmul(ckv_ps[:], hid_sb[:, kc, :], wdkv_t[:], start=st, stop=sp)
                    nc.tensor.matmul(kr_ps[:], hid_sb[:, kc, :], wkr_t[:], start=st, stop=sp)

                nc.vector.tensor_copy(out=cq_sb[:], in_=cq_ps[:])
                nc.vector.tensor_copy(out=ckv_sb[:], in_=ckv_ps[:])
                nc.sync.dma_start(ckv_new[:, :], ckv_sb[:])

                # RoPE on k_R (interleaved rotate-half at pos=CACHE_LEN)
                rt = p0c.tile([4, 4, 32], F32)
                nc.vector.tensor_tensor(rt[:, 0, :], kr_ps[:, 0::2], cosk_sb[:], OP.mult)
                nc.vector.tensor_tensor(rt[:, 1, :], kr_ps[:, 1::2], sink_sb[:], OP.mult)
                nc.vector.tensor_tensor(kr_sb[:, 0::2], rt[:, 0, :], rt[:, 1, :], OP.subtract)
                nc.vector.tensor_tensor(rt[:, 2, :], kr_ps[:, 0::2], sink_sb[:], OP.mult)
                nc.vector.tensor_tensor(rt[:, 3, :], kr_ps[:, 1::2], cosk_sb[:], OP.mult)
                nc.vector.tensor_tensor(kr_sb[:, 1::2], rt[:, 2, :], rt[:, 3, :], OP.add)
                nc.sync.dma_start(kr_new[:, :], kr_sb[:])

                # transpose c_Q -> [1536, 4] and c_KV -> [512, 4]
                t_ps = p0ps.tile([128, 128], F32, tag="tp", name="t_ps")[:, 0:48]
                for t in range(12):
                    nc.tensor.transpose(t_ps[:, t * 4:(t + 1) * 4], cq_sb[:, t * 128:(t + 1) * 128], id4)
                nc.vector.tensor_copy(out=cqT_sb[:], in_=t_ps[:].rearrange("p (t b) -> p t b", b=4))
                t2_ps = p0ps.tile([128, 128], F32, tag="tp", name="t2_ps")[:, 0:16]
                for cb in range(4):
                    nc.tensor.transpose(t2_ps[:, cb * 4:(cb + 1) * 4], ckv_sb[:, cb * 128:(cb + 1) * 128], id4)
                nc.vector.tensor_copy(out=ckvT_sb[:], in_=t2_ps[:].rearrange("p (t b) -> p t b", b=4))

                # q_C = c_Q @ W_UQ_slice ; q_R = c_Q @ W_QR_slice
                qc_ps = p0ps.tile([4, 512], F32, tag="ckv")
                qr_ps = p0ps.tile([4, 256], F32, tag="kr")
                for t in range(12):
                    nc.tensor.matmul(qc_ps[:], cqT_sb[:, t, :], wuq_sb[:, t, :],
                                     start=(t == 0), stop=(t == 11))
                    nc.tensor.matmul(qr_ps[:], cqT_sb[:, t, :], wqr_sb[:, t, :],
                                     start=(t == 0), stop=(t == 11))
                nc.vector.tensor_copy(out=qc_sb[:], in_=qc_ps[:])

                # RoPE on q_R (per-head-tiled tables)
                rq = p0c.tile([4, 4, 128], F32)
                nc.vector.tensor_tensor(rq[:, 0, :], qr_ps[:, 0::2], cosq_sb[:], OP.mult)
                nc.vector.tensor_tensor(rq[:, 1, :], qr_ps[:, 1::2], sinq_sb[:], OP.mult)
                nc.vector.tensor_tensor(qr_sb[:, 0::2], rq[:, 0, :], rq[:, 1, :], OP.subtract)
                nc.vector.tensor_tensor(rq[:, 2, :], qr_ps[:, 0::2], sinq_sb[:], OP.mult)
                nc.vector.tensor_tensor(rq[:, 3, :], qr_ps[:, 1::2], cosq_sb[:], OP.mult)
                nc.vector.tensor_tensor(qr_sb[:, 1::2], rq[:, 2, :], rq[:, 3, :], OP.add)

                # q_C^T
                t3_ps = p0ps.tile([128, 128], F32, tag="tp", name="t3_ps")[:, 0:16]
                for h in range(4):
                    nc.tensor.transpose(t3_ps[:, h * 4:(h + 1) * 4], qc_sb[:, h * 128:(h + 1) * 128], id4)
                nc.vector.tensor_copy(out=qcT_sb[:], in_=t3_ps[:].rearrange("p (t b) -> p t b", b=4))

                # q_eff^T[c, b] per head: W_UK_h^T-chunks x q_C_h^T
                qe_ps = p0ps.tile([128, 128], F32, tag="tp", name="qe_ps")[:, 0:64]
                for h in range(4):
                    for mc in range(4):
                        nc.tensor.matmul(qe_ps[:, mc * 16 + h * 4: mc * 16 + h * 4 + 4],
                                         wukt_sb[:, h, mc * 128:(mc + 1) * 128],
                                         qcT_sb[:, h, :], start=True, stop=True)
                nc.vector.tensor_copy(out=qeffT_sb[:],
                                      in_=qe_ps[:].rearrange("p (cb h b) -> p cb h b", h=4, b=4))

                # q_R^T rows of the 5th score K-chunk
                t4_ps = p0ps.tile([128, 128], F32, tag="tp", name="t4_ps")[0:64, 0:16]
                for h in range(4):
                    nc.tensor.transpose(t4_ps[:, h * 4:(h + 1) * 4], qr_sb[:, h * 64:(h + 1) * 64], id4)
                nc.vector.tensor_copy(out=qaug4_sb[0:64, :, :],
                                      in_=t4_ps[:].rearrange("p (h b) -> p h b", b=4))

                # current-token K column: [k_R_roped^T ; mask]
                t5_ps = p0ps.tile([128, 128], F32, tag="tp", name="t5_ps")[0:64, 0:4]
                nc.tensor.transpose(t5_ps[:], kr_sb[:], id4)
                nc.vector.tensor_copy(out=kRcur_sb[0:64, :], in_=t5_ps[:])
                nc.sync.dma_start(kRcur_sb[64:65, :], mask_c[:, :])

                # current-token scores + exp into probs col 4096
                scur_ps = p0ps.tile([128, 128], F32, tag="tp", name="scur_ps")[:, 0:1]
                for b in range(4):
                    for kc in range(4):
                        nc.tensor.matmul(scur_ps[32 * b:32 * b + 4, :],
                                         qeffT_sb[:, kc, :, b], ckvT_sb[:, kc, b:b + 1],
                                         start=(kc == 0), stop=False,
                                         tile_position=(0, 32 * b))
                    nc.tensor.matmul(scur_ps[32 * b:32 * b + 4, :],
                                     qaug4_sb[:, :, b], kRcur_sb[:, b:b + 1],
                                     start=False, stop=True, tile_position=(0, 32 * b))
                    nc.scalar.activation(out=probs_sb[32 * b:32 * b + 4, 4096:4097],
                                         in_=scur_ps[32 * b:32 * b + 4, :],
                                         func=AF.Exp, scale=SCALE)
                # transpose current-token probs -> [1, 128] (rows 1-3 zero-padded)
                t6_ps = p0ps.tile([128, 128], F32, tag="tp", name="t6_ps")
                nc.tensor.transpose(t6_ps[:], probs_sb[:, 4096:4224], id_sb[:])
                nc.vector.tensor_copy(out=pT33_sb[:], in_=t6_ps[0:4, :])

                # current-token V rows (row 0 of each batch's K=4-padded tile)
                for b in range(4):
                    nc.sync.dma_start(ccur_sb[0:1, b, :], ckv_sb[b:b + 1, :])

            # ---------------- Main loop: scores + PV over the cache --------
            with tc.tile_pool(name="mc", bufs=2) as cpool, \
                 tc.tile_pool(name="ma", bufs=2) as apool, \
                 tc.tile_pool(name="mt", bufs=2) as ctpool, \
                 tc.tile_pool(name="mp", bufs=2) as ptpool, \
                 tc.tile_pool(name="mw", bufs=3) as wopool, \
                 tc.tile_pool(name="mps", bufs=1, space="PSUM") as mps:
                ct_ps = mps.tile([128, 2048], F32)
                sc_psA = mps.tile([128, 512], F32, tag="scA")
                sc_psB = mps.tile([128, 512], F32, tag="scB")
                pT_ps = mps.tile([128, 512], F32, tag="pT")
                attn_ps = mps.tile([128, 512], F32, tag="attn")
                nc.vector.memset(sc_psA[:], 0.0)
                nc.vector.memset(sc_psB[:], 0.0)
                nc.vector.memset(attn_ps[:], 0.0)

                for sc in range(NSUP):
                    scps = sc_psA if sc % 2 == 0 else sc_psB
                    csb = []
                    aug = []
                    for b in range(4):
                        ct = cpool.tile([128, 4, 512], F32, tag=f"c{b}")
                        nc.sync.dma_start(
                            ct[:], ckv_c[b, sc * 512:(sc + 1) * 512, :].rearrange(
                                "(ss p) c -> p ss c", p=128))
                        csb.append(ct)
                        at = apool.tile([65, 512], F32, tag=f"a{b}")
                        nc.sync.dma_start(at[0:64, :], krT[b, :, sc * 512:(sc + 1) * 512])
                        nc.sync.dma_start(at[64:65, :], mask_s[b:b + 1, sc * 512:(sc + 1) * 512])
                        aug.append(at)
                    for b in range(4):
                        ctsb = ctpool.tile([128, 4, 512], F32)
                        for cb in range(4):
                            for ss in range(4):
                                nc.tensor.transpose(
                                    ct_ps[:, cb * 512 + ss * 128: cb * 512 + (ss + 1) * 128],
                                    csb[b][:, ss, cb * 128:(cb + 1) * 128], id_sb[:])
                            nc.vector.tensor_copy(out=ctsb[:, cb, :],
                                                  in_=ct_ps[:, cb * 512:(cb + 1) * 512])
                        for kc in range(4):
                            nc.tensor.matmul(scps[32 * b:32 * b + 4, :],
                                             qeffT_sb[:, kc, :, b], ctsb[:, kc, :],
                                             start=(kc == 0), stop=False,
                                             tile_position=(0, 32 * b))
                        nc.tensor.matmul(scps[32 * b:32 * b + 4, :],
                                         qaug4_sb[:, :, b], aug[b][:],
                                         start=False, stop=True, tile_position=(0, 32 * b))
                    nc.scalar.activation(out=probs_sb[:, sc * 512:(sc + 1) * 512],
                                         in_=scps[:], func=AF.Exp, scale=SCALE)
                    for ss in range(4):
                        nc.tensor.transpose(pT_ps[:, ss * 128:(ss + 1) * 128],
                                            probs_sb[:, (4 * sc + ss) * 128:(4 * sc + ss + 1) * 128],
                                            id_sb[:])
                    pTsb = ptpool.tile([128, 512], F32)
                    nc.vector.tensor_copy(out=pTsb[:], in_=pT_ps[:])
                    for b in range(4):
                        for ss in range(4):
                            nc.tensor.matmul(attn_ps[32 * b:32 * b + 4, :],
                                             pTsb[:, ss * 128 + 32 * b: ss * 128 + 32 * b + 4],
                                             csb[b][:, ss, :],
                                             start=(sc == 0 and ss == 0), stop=False,
                                             tile_position=(0, 32 * b))

                # current token closes each batch's PV accumulation
                for b in range(4):
                    nc.tensor.matmul(attn_ps[32 * b:32 * b + 4, :],
                                     pT33_sb[:, 32 * b:32 * b + 4], ccur_sb[:, b, :],
                                     start=False, stop=True, tile_position=(0, 32 * b))

                # ---------------- Normalize + output projection ------------
                nc.vector.reduce_sum(out=den_sb[:], in_=probs_sb[:], axis=mybir.AxisListType.X)
                nc.vector.reciprocal(out=recip_sb[:], in_=den_sb[:])
                nc.vector.tensor_scalar_mul(out=attn_sb[:], in0=attn_ps[:], scalar1=recip_sb[:])

                for cb in range(4):
                    nc.tensor.transpose(pT_ps[:, cb * 128:(cb + 1) * 128],
                                        attn_sb[:, cb * 128:(cb + 1) * 128], id_sb[:])
                nc.vector.tensor_copy(out=attnT_sb[:],
                                      in_=pT_ps[:].rearrange("p (cb b h) -> p cb b h", cb=4, b=4))

                wuv_sb = wopool.tile([128, 4, 512], F32, tag="wuv")
                nc.sync.dma_start(wuv_sb[:], wuv_s[:, :].rearrange("(kc p) n -> p kc n", p=128))
                for h in range(4):
                    for cb in range(4):
                        nc.tensor.matmul(sc_psA[:, h * 4:(h + 1) * 4],
                                         wuv_sb[:, cb, h * 128:(h + 1) * 128],
                                         attnT_sb[:, cb, :, h],
                                         start=(cb == 0), stop=(cb == 3))
                nc.vector.tensor_copy(out=ohT_sb[:],
                                      in_=sc_psA[:, 0:16].rearrange("p (h b) -> p h b", b=4))

                for n in range(8):
                    wot = wopool.tile([128, 4, 512], F32, tag="wo")
                    nc.sync.dma_start(
                        wot[:], wo_s[:, :].rearrange("(kb p) n -> p kb n", p=128)[:, :, n * 512:(n + 1) * 512])
                    for kb in range(4):
                        nc.tensor.matmul(sc_psB[0:4, :], ohT_sb[:, kb, :], wot[:, kb, :],
                                         start=(kb == 0), stop=(kb == 3))
                    nc.vector.tensor_copy(out=outp_sb[:, n * 512:(n + 1) * 512], in_=sc_psB[0:4, :])
                nc.sync.dma_start(out_p[:, :], outp_sb[:])

    nc.finalize()
    return nc


_NC_CACHE = {}


def _get_nc():
    if "nc" not in _NC_CACHE:
        _NC_CACHE["nc"] = _build_nc()
    return _NC_CACHE["nc"]


def _host_prep(inputs):
    hidden = np.asarray(inputs["hidden_states"], dtype=np.float32)
    mask = np.asarray(inputs["mask"], dtype=np.float32)
    cached_c_KV = np.asarray(inputs["cached_c_KV"], dtype=np.float32)
    cached_k_R = np.asarray(inputs["cached_k_R"], dtype=np.float32)
    W_DKV = np.asarray(inputs["W_DKV"], dtype=np.float32)
    W_UK = np.asarray(inputs["W_UK"], dtype=np.float32)
    W_UV = np.asarray(inputs["W_UV"], dtype=np.float32)
    W_DQ = np.asarray(inputs["W_DQ"], dtype=np.float32)
    W_UQ = np.asarray(inputs["W_UQ"], dtype=np.float32)
    W_QR = np.asarray(inputs["W_QR"], dtype=np.float32)
    W_KR = np.asarray(inputs["W_KR"], dtype=np.float32)
    W_O = np.asarray(inputs["W_O"], dtype=np.float32)

    hT = hidden[:, 0, :].T                                  # [4096, 4]
    hid_sw = np.ascontiguousarray(
        hT.reshape(32, 128, 4).transpose(1, 0, 2).reshape(128, 128))
    krT = np.ascontiguousarray(cached_k_R.transpose(0, 2, 1))
    mask2 = mask[:, 0, 0, :]                                # [4, 4097]
    mask_sc = np.ascontiguousarray(mask2 * (-1e9 / SCALE)).astype(np.float32)
    mask_cc = np.ascontiguousarray(mask_sc[:, CACHE_LEN].reshape(1, BATCH))

    inv = (1.0 / (10000.0 ** (np.arange(0, ROPE_DIM, 2, dtype=np.float64) / ROPE_DIM)))
    ang = float(CACHE_LEN) * inv
    cos = np.cos(ang).astype(np.float32)                    # [32]
    sin = np.sin(ang).astype(np.float32)
    cos_k = np.tile(cos, (BATCH, 1))
    sin_k = np.tile(sin, (BATCH, 1))
    cos_q = np.tile(np.tile(cos, H_LOC), (BATCH, 1))
    sin_q = np.tile(np.tile(sin, H_LOC), (BATCH, 1))
    ident = np.eye(128, dtype=np.float32)

    shared = {
        "hid_sw": hid_sw, "ckv_c": cached_c_KV, "krT": krT,
        "mask_s": mask_sc, "mask_c": mask_cc,
        "wdkv": W_DKV, "wdq": W_DQ, "wkr": W_KR,
        "cos_q": cos_q, "sin_q": sin_q, "cos_k": cos_k, "sin_k": sin_k,
        "ident": ident,
    }
    in_maps = []
    for g in range(N_CORES):
        m = dict(shared)
        m["wuq_s"] = np.ascontiguousarray(W_UQ[:, g * 512:(g + 1) * 512])
        m["wqr_s"] = np.ascontiguousarray(W_QR[:, g * 256:(g + 1) * 256])
        m["wukt_s"] = np.ascontiguousarray(W_UK[:, g * 512:(g + 1) * 512].T)
        m["wuv_s"] = np.ascontiguousarray(W_UV[:, g * 512:(g + 1) * 512])
        m["wo_s"] = np.ascontiguousarray(W_O[g * 512:(g + 1) * 512, :])
        in_maps.append(m)
    return in_maps, cached_c_KV, cached_k_R


def _assemble(results, cached_c_KV, cached_k_R):
    out = np.zeros((BATCH, HIDDEN), dtype=np.float64)
    for r in results:
        out += r["out_p"].astype(np.float64)
    out = out.astype(np.float32).reshape(BATCH, 1, HIDDEN)
    ckv_row = results[0]["ckv_new"][:, None, :]             # [4, 1, 512]
    kr_row = results[0]["kr_new"][:, None, :]               # [4, 1, 64]
    new_c_KV = np.concatenate([cached_c_KV, ckv_row], axis=1)
    new_k_R = np.concatenate([cached_k_R, kr_row], axis=1)
    return out, new_c_KV, new_k_R


def run(inputs, **spmd_kwargs):
    """Run on 8 cores; returns ((out, new_c_KV, new_k_R), BassKernelResults)."""
    nc = _get_nc()
    in_maps, cached_c_KV, cached_k_R = _host_prep(inputs)
    res = run_bass_kernel_spmd(nc, in_maps, core_ids=list(range(N_CORES)), **spmd_kwargs)
    return _assemble(res.results, cached_c_KV, cached_k_R), res


def kernel(**inputs):
    (out, new_c_KV, new_k_R), _ = run(inputs)
    return out, new_c_KV, new_k_R
